# revision 10
# baseline (speedup 1.0000x reference)
"""LMS adaptive filter (BaseFilter) on 8 TRN2 NeuronCores.

Algorithm per (batch b, frame f): 64-tap LMS over 416 sequential steps.
  e_t   = d[b, 256f + 32 + t] - sum_k w[k] * x[256f + t + k]
  w    += MU * e_t * x[256f + t : +64]
Outputs (overlap-add): d_est = d - e and e, assembled per reference.

Sharding: 4096 frames split 512/core (both batches on every core) ->
1024 independent sequences/core = 8 groups x 128 partitions.

Inner loop = 3 fused DVE instructions per step per group
(tensor_tensor_reduce crashes the exec unit on this runtime, so the dot
product uses scalar_tensor_tensor's accum_out instead):
  scalar_tensor_tensor : prod = (w * -1) * x_win ; ns = sum(prod)
  tensor_scalar        : e_t  = ns + d_t
  scalar_tensor_tensor : w    = (mu*x_win) * e_t + w      (in-place)
  tensor_scalar        : w    = clip(w, +-65535)  -- the recursion is
                         unstable (mu*|x|^2 ~ 3.2 > 2) so w rides the
                         clip rails; the clip is essential
"""

import numpy as np

HOP = 256
FRAMELEN = 512
K = 64
WD = 32
MU = 0.05
WMIN, WMAX = -65535.0, 65535.0
B = 2
F = 4096
NC = 8
F_LOC = F // NC              # 512 frames per core
S = (FRAMELEN - K) - WD      # 416 sequential steps
TSTART = (FRAMELEN - HOP) - WD  # 224: first step kept for frames >= 1
TAIL = S - TSTART            # 192 output elements per frame >= 1
SPAN = HOP * (F_LOC - 1) + FRAMELEN  # 131328: x/d elements per core shard
CORE_STRIDE = HOP * F_LOC    # 131072
OUT_LEN = (FRAMELEN - K) + (F - 1) * TAIL  # 786688

_CACHE = {}


def _build():
    import concourse.bacc as bacc
    import concourse.tile as tile
    from concourse import mybir
    import concourse.bass as bass

    f32 = mybir.dt.float32
    AluOp = mybir.AluOpType

    nc = bacc.Bacc("TRN2", target_bir_lowering=False)
    x_in = nc.dram_tensor("x", [SPAN], f32, kind="ExternalInput")
    d_in = nc.dram_tensor("d", [B, SPAN], f32, kind="ExternalInput")
    # [kind(0=d_est,1=e)][b][f_local][j] , j <-> step t = TSTART + j
    out_main = nc.dram_tensor("out_main", [2, B, F_LOC, TAIL], f32,
                              kind="ExternalOutput")
    # frame 0 of this core: steps t < TSTART   [kind][b][t]
    out_head = nc.dram_tensor("out_head", [2, B, TSTART], f32,
                              kind="ExternalOutput")

    with tile.TileContext(nc) as tc:
        with tc.tile_pool(name="p", bufs=1) as pool:
            XF = pool.tile([128, 4, FRAMELEN], f32)    # x frames (slab fg)
            XFMU = pool.tile([128, 4, FRAMELEN], f32)  # MU * x frames
            DB = pool.tile([128, B, 4, S], f32)        # d at step offsets
            W = [pool.tile([128, K], f32, name=f"W{g}", tag=f"w{g}") for g in range(8)]
            EB = [pool.tile([128, S], f32, name=f"EB{g}", tag=f"e{g}") for g in range(8)]
            PROD = [pool.tile([128, K], f32, name=f"PROD{g}", tag=f"pr{g}") for g in range(8)]
            NS = [pool.tile([128, 1], f32, name=f"NS{g}", tag=f"ns{g}") for g in range(8)]
            DEST = [pool.tile([128, S], f32, name=f"DEST{g}", tag=f"de{g}") for g in range(8)]

            # partition p, slab fg  ->  frame f_local = fg*128 + p
            nc.sync.dma_start(
                XF[:],
                bass.AP(tensor=x_in, offset=0,
                        ap=[[HOP, 128], [HOP * 128, 4], [1, FRAMELEN]]),
            )
            for b in range(B):
                nc.sync.dma_start(
                    DB[:, b, :, :],
                    bass.AP(tensor=d_in, offset=b * SPAN + WD,
                            ap=[[HOP, 128], [HOP * 128, 4], [1, S]]),
                )
            nc.vector.tensor_scalar_mul(XFMU[:], XF[:], MU)
            for g in range(8):
                nc.vector.memset(W[g][:], 0.0)

            for t in range(S):
                for g in range(8):
                    b, fg = divmod(g, 4)
                    nc.vector.scalar_tensor_tensor(
                        out=PROD[g][:], in0=W[g][:], scalar=-1.0,
                        in1=XF[:, fg, t:t + K],
                        op0=AluOp.mult, op1=AluOp.mult,
                        accum_out=NS[g][:, 0:1],
                    )
                    nc.vector.tensor_scalar(
                        out=EB[g][:, t:t + 1], in0=NS[g][:, 0:1],
                        scalar1=DB[:, b, fg, t:t + 1], scalar2=None,
                        op0=AluOp.add,
                    )
                    nc.vector.scalar_tensor_tensor(
                        out=W[g][:], in0=XFMU[:, fg, t:t + K],
                        scalar=EB[g][:, t:t + 1], in1=W[g][:],
                        op0=AluOp.mult, op1=AluOp.add,
                    )
                    nc.vector.tensor_scalar(
                        out=W[g][:], in0=W[g][:],
                        scalar1=WMAX, scalar2=WMIN,
                        op0=AluOp.min, op1=AluOp.max,
                    )

            # d_est = d - e
            for g in range(8):
                b, fg = divmod(g, 4)
                nc.vector.tensor_sub(DEST[g][:], DB[:, b, fg, :], EB[g][:])

            # outputs: frames >= 1 use steps [TSTART, S); f_local = fg*128+p
            for g in range(8):
                b, fg = divmod(g, 4)
                for kind, src in ((0, DEST[g]), (1, EB[g])):
                    nc.sync.dma_start(
                        bass.AP(tensor=out_main,
                                offset=(kind * B + b) * F_LOC * TAIL
                                + fg * 128 * TAIL,
                                ap=[[TAIL, 128], [1, TAIL]]),
                        src[:, TSTART:S],
                    )
            # head: local frame 0 = (fg=0, p=0) -> groups 0 (b=0) and 4 (b=1)
            for b in range(B):
                g = b * 4
                for kind, src in ((0, DEST[g]), (1, EB[g])):
                    nc.sync.dma_start(
                        bass.AP(tensor=out_head,
                                offset=(kind * B + b) * TSTART,
                                ap=[[TSTART, 1], [1, TSTART]]),
                        src[0:1, 0:TSTART],
                    )
    nc.finalize()
    return nc


def _get_nc():
    if "nc" not in _CACHE:
        _CACHE["nc"] = _build()
    return _CACHE["nc"]


def run_shards(d, x, trace=False, **kw):
    from concourse.bass_utils import run_bass_kernel_spmd

    nc = _get_nc()
    in_maps = []
    for c in range(NC):
        lo = c * CORE_STRIDE
        in_maps.append({
            "x": np.ascontiguousarray(x[lo:lo + SPAN], dtype=np.float32),
            "d": np.ascontiguousarray(d[:, lo:lo + SPAN], dtype=np.float32),
        })
    return run_bass_kernel_spmd(nc, in_maps, core_ids=list(range(NC)),
                                trace=trace, **kw)


def assemble(results):
    mains = np.stack([r["out_main"] for r in results])  # (8, 2, B, 512, 192)
    head = results[0]["out_head"]                       # (2, B, 224)
    outs = []
    for kind in range(2):
        m = mains[:, kind].transpose(1, 0, 2, 3).reshape(B, F, TAIL)
        o = np.zeros((B, OUT_LEN), np.float32)
        o[:, WD:WD + TSTART] = head[kind]
        o[:, WD + TSTART:FRAMELEN - K] = m[:, 0]
        o[:, FRAMELEN - K:] = m[:, 1:].reshape(B, -1)
        outs.append(o)
    return outs[0], outs[1]


def kernel(d, x):
    res = run_shards(d, x)
    return assemble(res.results)


# revision 11
# speedup vs baseline: 1.5788x; 1.5788x over previous
"""LMS adaptive filter (BaseFilter) on 8 TRN2 NeuronCores.

Algorithm per (batch b, frame f): 64-tap LMS over 416 sequential steps.
  e_t   = d[b, 256f + 32 + t] - sum_k w[k] * x[256f + t + k]
  w    += MU * e_t * x[256f + t : +64]
Outputs (overlap-add): d_est = d - e and e, assembled per reference.

Sharding: 4096 frames split 512/core (both batches on every core) ->
1024 independent sequences/core = 8 groups x 128 partitions.

Inner loop = 3 fused DVE instructions per step per group
(tensor_tensor_reduce crashes the exec unit on this runtime, so the dot
product uses scalar_tensor_tensor's accum_out instead):
  scalar_tensor_tensor : prod = (w * -1) * x_win ; ns = sum(prod)
  tensor_scalar        : e_t  = ns + d_t
  scalar_tensor_tensor : w    = (mu*x_win) * e_t + w      (in-place)
  tensor_scalar        : w    = clip(w, +-65535)  -- the recursion is
                         unstable (mu*|x|^2 ~ 3.2 > 2) so w rides the
                         clip rails; the clip is essential
"""

import numpy as np

HOP = 256
FRAMELEN = 512
K = 64
WD = 32
MU = 0.05
WMIN, WMAX = -65535.0, 65535.0
B = 2
F = 4096
NC = 8
F_LOC = F // NC              # 512 frames per core
S = (FRAMELEN - K) - WD      # 416 sequential steps
TSTART = (FRAMELEN - HOP) - WD  # 224: first step kept for frames >= 1
TAIL = S - TSTART            # 192 output elements per frame >= 1
SPAN = HOP * (F_LOC - 1) + FRAMELEN  # 131328: x/d elements per core shard
CORE_STRIDE = HOP * F_LOC    # 131072
OUT_LEN = (FRAMELEN - K) + (F - 1) * TAIL  # 786688

_CACHE = {}


def _build():
    import concourse.bacc as bacc
    import concourse.tile as tile
    from concourse import mybir
    import concourse.bass as bass

    f32 = mybir.dt.float32
    AluOp = mybir.AluOpType

    nc = bacc.Bacc("TRN2", target_bir_lowering=False)
    x_in = nc.dram_tensor("x", [SPAN], f32, kind="ExternalInput")
    d_in = nc.dram_tensor("d", [B, SPAN], f32, kind="ExternalInput")
    # [kind(0=d_est,1=e)][b][f_local][j] , j <-> step t = TSTART + j
    out_main = nc.dram_tensor("out_main", [2, B, F_LOC, TAIL], f32,
                              kind="ExternalOutput")
    # frame 0 of this core: steps t < TSTART   [kind][b][t]
    out_head = nc.dram_tensor("out_head", [2, B, TSTART], f32,
                              kind="ExternalOutput")

    with tile.TileContext(nc) as tc:
        with tc.tile_pool(name="p", bufs=1) as pool:
            XF = pool.tile([128, 4, FRAMELEN], f32)    # x frames (slab fg)
            XFMU = pool.tile([128, 4, FRAMELEN], f32)  # MU * x frames
            DB = pool.tile([128, B, 4, S], f32)        # d at step offsets
            W = [pool.tile([128, K], f32, name=f"W{g}", tag=f"w{g}") for g in range(8)]
            EB = [pool.tile([128, S], f32, name=f"EB{g}", tag=f"e{g}") for g in range(8)]
            PROD = [pool.tile([128, K], f32, name=f"PROD{g}", tag=f"pr{g}") for g in range(8)]
            NS = [pool.tile([128, 1], f32, name=f"NS{g}", tag=f"ns{g}") for g in range(8)]
            DEST = [pool.tile([128, S], f32, name=f"DEST{g}", tag=f"de{g}") for g in range(8)]

            # partition p, slab fg  ->  frame f_local = fg*128 + p
            nc.sync.dma_start(
                XF[:],
                bass.AP(tensor=x_in, offset=0,
                        ap=[[HOP, 128], [HOP * 128, 4], [1, FRAMELEN]]),
            )
            for b in range(B):
                nc.sync.dma_start(
                    DB[:, b, :, :],
                    bass.AP(tensor=d_in, offset=b * SPAN + WD,
                            ap=[[HOP, 128], [HOP * 128, 4], [1, S]]),
                )
            nc.vector.tensor_scalar_mul(XFMU[:], XF[:], MU)
            for g in range(8):
                nc.vector.memset(W[g][:], 0.0)

            # per-engine emission order: DVE streams dots then updates;
            # ScalarE computes e; GpSimd clips. Round-robin over the 8
            # independent groups hides the cross-engine latency.
            for t in range(S):
                for g in range(8):
                    b, fg = divmod(g, 4)
                    nc.vector.scalar_tensor_tensor(
                        out=PROD[g][:], in0=W[g][:], scalar=-1.0,
                        in1=XF[:, fg, t:t + K],
                        op0=AluOp.mult, op1=AluOp.mult,
                        accum_out=NS[g][:, 0:1],
                    )
                for g in range(8):
                    b, fg = divmod(g, 4)
                    nc.scalar.activation(
                        out=EB[g][:, t:t + 1], in_=NS[g][:, 0:1],
                        func=mybir.ActivationFunctionType.Identity,
                        bias=DB[:, b, fg, t:t + 1], scale=1.0,
                    )
                for g in range(8):
                    b, fg = divmod(g, 4)
                    nc.vector.scalar_tensor_tensor(
                        out=W[g][:], in0=XFMU[:, fg, t:t + K],
                        scalar=EB[g][:, t:t + 1], in1=W[g][:],
                        op0=AluOp.mult, op1=AluOp.add,
                    )
                for g in range(8):
                    nc.gpsimd.tensor_scalar(
                        out=W[g][:], in0=W[g][:],
                        scalar1=WMAX, scalar2=WMIN,
                        op0=AluOp.min, op1=AluOp.max,
                    )

            # d_est = d - e
            for g in range(8):
                b, fg = divmod(g, 4)
                nc.vector.tensor_sub(DEST[g][:], DB[:, b, fg, :], EB[g][:])

            # outputs: frames >= 1 use steps [TSTART, S); f_local = fg*128+p
            for g in range(8):
                b, fg = divmod(g, 4)
                for kind, src in ((0, DEST[g]), (1, EB[g])):
                    nc.sync.dma_start(
                        bass.AP(tensor=out_main,
                                offset=(kind * B + b) * F_LOC * TAIL
                                + fg * 128 * TAIL,
                                ap=[[TAIL, 128], [1, TAIL]]),
                        src[:, TSTART:S],
                    )
            # head: local frame 0 = (fg=0, p=0) -> groups 0 (b=0) and 4 (b=1)
            for b in range(B):
                g = b * 4
                for kind, src in ((0, DEST[g]), (1, EB[g])):
                    nc.sync.dma_start(
                        bass.AP(tensor=out_head,
                                offset=(kind * B + b) * TSTART,
                                ap=[[TSTART, 1], [1, TSTART]]),
                        src[0:1, 0:TSTART],
                    )
    nc.finalize()
    return nc


def _get_nc():
    if "nc" not in _CACHE:
        _CACHE["nc"] = _build()
    return _CACHE["nc"]


def run_shards(d, x, trace=False, **kw):
    from concourse.bass_utils import run_bass_kernel_spmd

    nc = _get_nc()
    in_maps = []
    for c in range(NC):
        lo = c * CORE_STRIDE
        in_maps.append({
            "x": np.ascontiguousarray(x[lo:lo + SPAN], dtype=np.float32),
            "d": np.ascontiguousarray(d[:, lo:lo + SPAN], dtype=np.float32),
        })
    return run_bass_kernel_spmd(nc, in_maps, core_ids=list(range(NC)),
                                trace=trace, **kw)


def assemble(results):
    mains = np.stack([r["out_main"] for r in results])  # (8, 2, B, 512, 192)
    head = results[0]["out_head"]                       # (2, B, 224)
    outs = []
    for kind in range(2):
        m = mains[:, kind].transpose(1, 0, 2, 3).reshape(B, F, TAIL)
        o = np.zeros((B, OUT_LEN), np.float32)
        o[:, WD:WD + TSTART] = head[kind]
        o[:, WD + TSTART:FRAMELEN - K] = m[:, 0]
        o[:, FRAMELEN - K:] = m[:, 1:].reshape(B, -1)
        outs.append(o)
    return outs[0], outs[1]


def kernel(d, x):
    res = run_shards(d, x)
    return assemble(res.results)
